# revision 13
# baseline (speedup 1.0000x reference)
"""Trainium2 Bass kernel for Compute1AngleInput (GNN angular message passing).

Strategy: data-parallel over the center-atom dimension across 8 NeuronCores.
Each core processes 1280 centers (10000 padded to 10240) in 10 tiles of 128
centers (one center per SBUF partition). Per tile:
  - direct DMA of neighbor indices / distances / center indices,
  - indirect-DMA gather of packed per-atom rows [x, y, z, species_bits],
  - indirect-DMA gather of embedding rows keyed by the gathered species,
  - DVE/ACT compute of the pair-distance grid d_jk and the normalized
    angular feature, then broadcast-assembly of the (128, 56*195) output
    tile in SBUF using strided/broadcast access patterns,
  - one large contiguous HWDGE store per tile (5.6 MB) to HBM.
The 437 MB output write is the roofline; everything else overlaps it.
"""

import numpy as np

import concourse.bacc as bacc
import concourse.bass as bass
import concourse.mybir as mybir
import concourse.tile as tile
from concourse.bass_utils import run_bass_kernel_spmd

N_CORES = 8
N_ATOMS = 50000
N_CENTER = 10000
NB = 8                 # neighbors
F = 64                 # embedding features
NPAIR = NB * (NB - 1)  # 56 off-diagonal (j, k) pairs
FEAT = 3 + 3 * F       # 195 features per pair
ROW = NPAIR * FEAT     # 10920 floats per center
P = 128                # SBUF partitions (centers per tile)
NCL = 1280             # centers per core (padded)
NT = NCL // P          # tiles per core
NPAD = N_CORES * NCL   # 10240

f32 = mybir.dt.float32
i32 = mybir.dt.int32
ALU = mybir.AluOpType


def _ap_of(base: bass.AP, off: int, dims) -> bass.AP:
    """View into an SBUF tile: keep the partition dim, custom free dims."""
    return bass.AP(
        base.tensor, base.offset + off, [list(base.ap[0])] + [list(d) for d in dims]
    )


def _build_body(nc, tc, tj_d, dd_d, ai_d, pk_d, em_d, bd_d, cst_d, out_d):
    with (
        tc.tile_pool(name="const", bufs=1) as cp,
        tc.tile_pool(name="io", bufs=3) as iop,
        tc.tile_pool(name="gath", bufs=3) as gp,
        tc.tile_pool(name="work", bufs=2) as wp,
        tc.tile_pool(name="psum", bufs=2, space="PSUM") as pp,
        tc.tile_pool(name="big", bufs=2) as bp,
    ):
        # constants: embed table, block-diag embed table, identity, iota16
        em_sb = cp.tile([16, F], f32)
        bd_sb = cp.tile([P, NB * F], f32)
        cst_sb = cp.tile([P, P + 1], f32)
        nc.scalar.dma_start(out=em_sb[:], in_=em_d[:, :])
        nc.scalar.dma_start(out=bd_sb[:], in_=bd_d[:, :])
        nc.scalar.dma_start(out=cst_sb[:], in_=cst_d[:, :])
        ident = cst_sb[:, 0:P]
        iota16 = cst_sb[:, P : P + 1]

        for t in range(NT):
            r0 = t * P
            tj = iop.tile([P, NB], i32, tag="tj")
            dd = iop.tile([P, NB], f32, tag="dd")
            ai = iop.tile([P, 1], i32, tag="ai")
            nc.scalar.dma_start(out=tj[:], in_=tj_d[r0 : r0 + P, :])
            nc.scalar.dma_start(out=dd[:], in_=dd_d[r0 : r0 + P, :])
            nc.scalar.dma_start(out=ai[:], in_=ai_d[r0 : r0 + P, :])

            # gather packed atom rows [x, y, z, species_f32]: one indirect DMA
            # per neighbor (HW supports one offset per partition per gather)
            xs = gp.tile([P, NB * 4], f32, tag="xs")
            si = gp.tile([P, 4], f32, tag="si")
            for j in range(NB):
                nc.gpsimd.indirect_dma_start(
                    out=xs[:, 4 * j : 4 * j + 4], out_offset=None, in_=pk_d[:, :],
                    in_offset=bass.IndirectOffsetOnAxis(ap=tj[:, j : j + 1], axis=0),
                )
            nc.gpsimd.indirect_dma_start(
                out=si[:], out_offset=None, in_=pk_d[:, :],
                in_offset=bass.IndirectOffsetOnAxis(ap=ai[:], axis=0),
            )

            # species -> embeddings via PE one-hot matmuls:
            # transpose species into (j*16+s, c) layout, compare each 16-row
            # band against iota16 -> one-hot lhsT, then lhsT.T @ blockdiag.
            pT = pp.tile([P, P], f32, tag="pT")
            spec_rep = bass.AP(
                xs[:].tensor, xs[:].offset + 3, [list(xs[:].ap[0]), [4, NB], [0, 16]]
            )
            s2 = wp.tile([P, P], f32, tag="s2")
            nc.vector.tensor_copy(out=s2[:].rearrange("p (j s) -> p j s", s=16), in_=spec_rep)
            nc.tensor.transpose(out=pT[:], in_=s2[:], identity=ident)
            lhsT = wp.tile([P, P], f32, tag="lhsT")
            nc.vector.tensor_scalar(
                out=lhsT[:], in0=pT[:], scalar1=iota16, scalar2=None,
                op0=ALU.is_equal,
            )
            pej = pp.tile([P, NB * F], f32, tag="pej")
            nc.tensor.matmul(out=pej[:], lhsT=lhsT[:], rhs=bd_sb[:], start=True, stop=True)

            pTi = pp.tile([16, P], f32, tag="pTi")
            s2i = wp.tile([P, 16], f32, tag="s2i")
            nc.vector.tensor_copy(out=s2i[:], in_=si[:, 3:4].broadcast_to((P, 16)))
            nc.tensor.transpose(out=pTi[:], in_=s2i[:], identity=ident)
            lhsTi = wp.tile([16, P], f32, tag="lhsTi")
            nc.vector.tensor_scalar(
                out=lhsTi[:], in0=pTi[:], scalar1=cst_sb[0:16, P : P + 1], scalar2=None,
                op0=ALU.is_equal,
            )
            pei = pp.tile([P, F], f32, tag="pei")
            nc.tensor.matmul(out=pei[:], lhsT=lhsTi[:], rhs=em_sb[:], start=True, stop=True)

            # ---- geometry: d_jk grid + normalized feature ----
            mn = wp.tile([P, 64], f32, tag="mn")
            mx = wp.tile([P, 64], f32, tag="mx")
            ss = wp.tile([P, 64], f32, tag="ss")
            tmp = wp.tile([P, 64], f32, tag="tmp")
            tmp2 = wp.tile([P, 64], f32, tag="tmp2")
            djk = wp.tile([P, 64], f32, tag="djk")
            normg = wp.tile([P, 64], f32, tag="normg")
            dkg = wp.tile([P, 64], f32, tag="dkg")
            rv = wp.tile([P, NB], f32, tag="rv")
            ejd = wp.tile([P, NB * F], f32, tag="ejd")

            dJ = dd[:].unsqueeze(2).broadcast_to((P, NB, NB))
            dK = dd[:].unsqueeze(1).broadcast_to((P, NB, NB))
            g3 = lambda a: a[:].rearrange("p (a b) -> p a b", b=NB)
            nc.vector.tensor_tensor(out=g3(mn), in0=dJ, in1=dK, op=ALU.min)
            nc.vector.tensor_tensor(out=g3(mx), in0=dJ, in1=dK, op=ALU.max)

            xs3 = xs[:].rearrange("p (j c) -> p j c", c=4)
            for ci in range(3):
                cJ = xs3[:, :, ci : ci + 1].broadcast_to((P, NB, NB))
                cK = xs3[:, :, ci : ci + 1].transpose((0, 2, 1)).broadcast_to((P, NB, NB))
                nc.vector.tensor_tensor(out=g3(tmp), in0=cJ, in1=cK, op=ALU.subtract)
                if ci == 0:
                    nc.vector.tensor_tensor(out=g3(ss), in0=g3(tmp), in1=g3(tmp), op=ALU.mult)
                else:
                    nc.vector.tensor_tensor(out=g3(tmp2), in0=g3(tmp), in1=g3(tmp), op=ALU.mult)
                    nc.vector.tensor_tensor(out=g3(ss), in0=g3(ss), in1=g3(tmp2), op=ALU.add)
            nc.scalar.sqrt(djk[:], ss[:])
            # norm = (d_jk - (mx - mn)) / (2 * mn)
            nc.vector.tensor_tensor(out=tmp[:], in0=mx[:], in1=mn[:], op=ALU.subtract)
            nc.vector.tensor_tensor(out=tmp2[:], in0=djk[:], in1=tmp[:], op=ALU.subtract)
            nc.vector.tensor_tensor(out=tmp[:], in0=mn[:], in1=mn[:], op=ALU.add)
            nc.vector.reciprocal(out=djk[:], in_=tmp[:])
            nc.vector.tensor_tensor(out=normg[:], in0=tmp2[:], in1=djk[:], op=ALU.mult)

            # d_ik grid materialized (for the off-diagonal copy)
            nc.scalar.copy(out=g3(dkg), in_=dK)

            # ejd = emb_j / d  (shared by the ej and ek blocks)
            nc.vector.reciprocal(out=rv[:], in_=dd[:])
            nc.vector.tensor_tensor(
                out=ejd[:].rearrange("p (j f) -> p j f", f=F),
                in0=pej[:].rearrange("p (j f) -> p j f", f=F),
                in1=rv[:].unsqueeze(2).broadcast_to((P, NB, F)),
                op=ALU.mult,
            )

            # ---- assemble the (128, 56*195) output tile ----
            big = bp.tile([P, ROW], f32, tag="big")
            bigap = big[:]
            # col0: d_ij (constant within each group of 7 pairs)
            nc.scalar.copy(
                out=_ap_of(bigap, 0, [[7 * FEAT, NB], [FEAT, 7]]),
                in_=dd[:].unsqueeze(2).broadcast_to((P, NB, 7)),
            )
            # col1: d_ik via off-diagonal view (flat[1:64] as (7,9)[:, :8])
            nc.scalar.copy(
                out=_ap_of(bigap, 1, [[8 * FEAT, 7], [FEAT, 8]]),
                in_=_ap_of(dkg[:], 1, [[9, 7], [1, 8]]),
            )
            # col2: d_jk_norm off-diagonal
            nc.vector.tensor_copy(
                out=_ap_of(bigap, 2, [[8 * FEAT, 7], [FEAT, 8]]),
                in_=_ap_of(normg[:], 1, [[9, 7], [1, 8]]),
            )
            # ei block: emb_i broadcast to all 56 pairs
            nc.vector.tensor_copy(
                out=_ap_of(bigap, 3, [[FEAT, NPAIR], [1, F]]),
                in_=_ap_of(pei[:], 0, [[0, NPAIR], [1, F]]),
            )
            # ej block: ejd[j] broadcast over the 7 pairs of group j
            nc.vector.tensor_copy(
                out=_ap_of(bigap, 67, [[7 * FEAT, NB], [FEAT, 7], [1, F]]),
                in_=_ap_of(ejd[:], 0, [[F, NB], [0, 7], [1, F]]),
            )
            # ek block: within group j, k runs over {0..7}\{j} as two runs
            for j in range(NB):
                if j > 0:
                    nc.scalar.copy(
                        out=_ap_of(bigap, (7 * j) * FEAT + 131, [[FEAT, j], [1, F]]),
                        in_=_ap_of(ejd[:], 0, [[F, j], [1, F]]),
                    )
                if j < 7:
                    nc.vector.tensor_copy(
                        out=_ap_of(bigap, (7 * j + j) * FEAT + 131, [[FEAT, 7 - j], [1, F]]),
                        in_=_ap_of(ejd[:], (j + 1) * F, [[F, 7 - j], [1, F]]),
                    )

            nc.sync.dma_start(out=out_d[r0 : r0 + P, :], in_=big[:])


_NC_CACHE = None


def _get_nc():
    global _NC_CACHE
    if _NC_CACHE is not None:
        return _NC_CACHE
    nc = bacc.Bacc("TRN2", target_bir_lowering=False, debug=False, num_devices=N_CORES)
    tj_d = nc.dram_tensor("tj", [NCL, NB], i32, kind="ExternalInput").ap()
    dd_d = nc.dram_tensor("dd", [NCL, NB], f32, kind="ExternalInput").ap()
    ai_d = nc.dram_tensor("ai", [NCL, 1], i32, kind="ExternalInput").ap()
    pk_d = nc.dram_tensor("pk", [N_ATOMS, 4], f32, kind="ExternalInput").ap()
    em_d = nc.dram_tensor("em", [16, F], f32, kind="ExternalInput").ap()
    bd_d = nc.dram_tensor("bd", [P, NB * F], f32, kind="ExternalInput").ap()
    cst_d = nc.dram_tensor("cst", [P, P + 1], f32, kind="ExternalInput").ap()
    out_d = nc.dram_tensor("ang", [NCL, ROW], f32, kind="ExternalOutput").ap()
    with tile.TileContext(nc) as tc:
        _build_body(nc, tc, tj_d, dd_d, ai_d, pk_d, em_d, bd_d, cst_d, out_d)
    nc.compile()
    _NC_CACHE = nc
    return nc


def kernel(nNeigh, atom_i_idx, atom_j_idx, dist_ij, atoms_xyz, atoms_long,
           embed_table, trace=False, tmpdir=None, **_unused):
    atom_i_idx = np.asarray(atom_i_idx)
    aj = np.asarray(atom_j_idx).astype(np.int32).reshape(N_CENTER, NB)
    dist = np.asarray(dist_ij).astype(np.float32).reshape(N_CENTER, NB)
    ai = np.asarray(atom_i_idx).astype(np.int32).reshape(N_CENTER, 1)
    xyz = np.asarray(atoms_xyz).astype(np.float32)
    spec = np.asarray(atoms_long)[:, 1].astype(np.int32)
    em = np.ascontiguousarray(np.asarray(embed_table).astype(np.float32))

    # packed per-atom table: [x, y, z, species (as float value)]
    pk = np.empty((N_ATOMS, 4), np.float32)
    pk[:, :3] = xyz
    pk[:, 3] = spec.astype(np.float32)

    # block-diagonal embed table (8 copies on the diagonal) for the one-hot
    # matmul, and [identity | iota16] constants
    bd = np.zeros((P, NB * F), np.float32)
    for j in range(NB):
        bd[16 * j : 16 * j + 16, F * j : F * j + F] = em
    cst = np.zeros((P, P + 1), np.float32)
    cst[:, :P] = np.eye(P, dtype=np.float32)
    cst[:, P] = np.arange(P, dtype=np.float32) % 16

    # pad the center dim to 8*1280 and shard
    def pad(a, fill):
        out = np.full((NPAD,) + a.shape[1:], fill, a.dtype)
        out[:N_CENTER] = a
        return out

    aj_p, dist_p, ai_p = pad(aj, 0), pad(dist, 1.0), pad(ai, 0)

    in_maps = []
    for c in range(N_CORES):
        s = slice(c * NCL, (c + 1) * NCL)
        in_maps.append({
            "tj": np.ascontiguousarray(aj_p[s]),
            "dd": np.ascontiguousarray(dist_p[s]),
            "ai": np.ascontiguousarray(ai_p[s]),
            "pk": pk,
            "em": em,
            "bd": bd,
            "cst": cst,
        })

    nc = _get_nc()
    res = run_bass_kernel_spmd(
        nc, in_maps, core_ids=list(range(N_CORES)), trace=trace, tmpdir=tmpdir
    )
    ang = np.concatenate([res.results[c]["ang"] for c in range(N_CORES)], axis=0)
    ang = ang[:N_CENTER].reshape(N_CENTER, NPAIR, FEAT)
    out = (atom_i_idx.reshape(-1), ang)
    if trace:
        return out, res
    return out


# revision 16
# speedup vs baseline: 1.0928x; 1.0928x over previous
"""Trainium2 Bass kernel for Compute1AngleInput (GNN angular message passing).

Strategy: data-parallel over the center-atom dimension across 8 NeuronCores.
Each core processes 1280 centers (10000 padded to 10240) in 10 tiles of 128
centers (one center per SBUF partition). Per tile:
  - direct DMA of neighbor indices / distances / center indices,
  - indirect-DMA gather of packed per-atom rows [x, y, z, species_bits],
  - indirect-DMA gather of embedding rows keyed by the gathered species,
  - DVE/ACT compute of the pair-distance grid d_jk and the normalized
    angular feature, then broadcast-assembly of the (128, 56*195) output
    tile in SBUF using strided/broadcast access patterns,
  - one large contiguous HWDGE store per tile (5.6 MB) to HBM.
The 437 MB output write is the roofline; everything else overlaps it.
"""

import numpy as np

import concourse.bacc as bacc
import concourse.bass as bass
import concourse.mybir as mybir
import concourse.tile as tile
from concourse.bass_utils import run_bass_kernel_spmd

N_CORES = 8
N_ATOMS = 50000
N_CENTER = 10000
NB = 8                 # neighbors
F = 64                 # embedding features
NPAIR = NB * (NB - 1)  # 56 off-diagonal (j, k) pairs
FEAT = 3 + 3 * F       # 195 features per pair
ROW = NPAIR * FEAT     # 10920 floats per center
P = 128                # SBUF partitions (centers per tile)
NCL = 1280             # centers per core (padded)
NT = NCL // P          # tiles per core
NPAD = N_CORES * NCL   # 10240

f32 = mybir.dt.float32
i32 = mybir.dt.int32
ALU = mybir.AluOpType


def _ap_of(base: bass.AP, off: int, dims) -> bass.AP:
    """View into an SBUF tile: keep the partition dim, custom free dims."""
    return bass.AP(
        base.tensor, base.offset + off, [list(base.ap[0])] + [list(d) for d in dims]
    )


def _build_body(nc, tc, tj_d, dd_d, ai_d, pk_d, em_d, bd_d, cst_d, out_d):
    with (
        tc.tile_pool(name="const", bufs=1) as cp,
        tc.tile_pool(name="io", bufs=4) as iop,
        tc.tile_pool(name="gath", bufs=4) as gp,
        tc.tile_pool(name="work", bufs=2) as wp,
        tc.tile_pool(name="psum", bufs=2, space="PSUM") as pp,
        tc.tile_pool(name="big", bufs=2) as bp,
    ):
        # constants: embed table, block-diag embed table, identity, iota16
        em_sb = cp.tile([16, F], f32)
        bd_sb = cp.tile([P, NB * F], f32)
        cst_sb = cp.tile([P, P + 1], f32)
        nc.scalar.dma_start(out=em_sb[:], in_=em_d[:, :])
        nc.scalar.dma_start(out=bd_sb[:], in_=bd_d[:, :])
        nc.scalar.dma_start(out=cst_sb[:], in_=cst_d[:, :])
        ident = cst_sb[:, 0:P]
        iota16 = cst_sb[:, P : P + 1]

        for t in range(NT):
            r0 = t * P
            tj = iop.tile([P, NB], i32, tag="tj")
            dd = iop.tile([P, NB], f32, tag="dd")
            ai = iop.tile([P, 1], i32, tag="ai")
            nc.scalar.dma_start(out=tj[:], in_=tj_d[r0 : r0 + P, :])
            nc.scalar.dma_start(out=dd[:], in_=dd_d[r0 : r0 + P, :])
            nc.scalar.dma_start(out=ai[:], in_=ai_d[r0 : r0 + P, :])

            # gather packed atom rows [x, y, z, species_f32]: one indirect DMA
            # per neighbor (HW supports one offset per partition per gather)
            xs = gp.tile([P, NB * 4], f32, tag="xs")
            si = gp.tile([P, 4], f32, tag="si")
            for j in range(NB):
                nc.gpsimd.indirect_dma_start(
                    out=xs[:, 4 * j : 4 * j + 4], out_offset=None, in_=pk_d[:, :],
                    in_offset=bass.IndirectOffsetOnAxis(ap=tj[:, j : j + 1], axis=0),
                )
            nc.gpsimd.indirect_dma_start(
                out=si[:], out_offset=None, in_=pk_d[:, :],
                in_offset=bass.IndirectOffsetOnAxis(ap=ai[:], axis=0),
            )

            # species -> embeddings via PE one-hot matmuls:
            # transpose species into (j*16+s, c) layout, compare each 16-row
            # band against iota16 -> one-hot lhsT, then lhsT.T @ blockdiag.
            pT = pp.tile([P, P], f32, tag="pT")
            spec_rep = bass.AP(
                xs[:].tensor, xs[:].offset + 3, [list(xs[:].ap[0]), [4, NB], [0, 16]]
            )
            s2 = wp.tile([P, P], f32, tag="s2")
            nc.scalar.copy(out=s2[:].rearrange("p (j s) -> p j s", s=16), in_=spec_rep)
            nc.tensor.transpose(out=pT[:], in_=s2[:], identity=ident)
            lhsT = wp.tile([P, P], f32, tag="lhsT")
            nc.vector.tensor_scalar(
                out=lhsT[:], in0=pT[:], scalar1=iota16, scalar2=None,
                op0=ALU.is_equal,
            )
            pej = pp.tile([P, NB * F], f32, tag="pej")
            nc.tensor.matmul(out=pej[:], lhsT=lhsT[:], rhs=bd_sb[:], start=True, stop=True)

            pTi = pp.tile([16, P], f32, tag="pTi")
            s2i = wp.tile([P, 16], f32, tag="s2i")
            nc.vector.tensor_copy(out=s2i[:], in_=si[:, 3:4].broadcast_to((P, 16)))
            nc.tensor.transpose(out=pTi[:], in_=s2i[:], identity=ident)
            lhsTi = wp.tile([16, P], f32, tag="lhsTi")
            nc.vector.tensor_scalar(
                out=lhsTi[:], in0=pTi[:], scalar1=cst_sb[0:16, P : P + 1], scalar2=None,
                op0=ALU.is_equal,
            )
            pei = pp.tile([P, F], f32, tag="pei")
            nc.tensor.matmul(out=pei[:], lhsT=lhsTi[:], rhs=em_sb[:], start=True, stop=True)

            # ---- geometry: d_jk grid + normalized feature ----
            mn = wp.tile([P, 64], f32, tag="mn")
            mx = wp.tile([P, 64], f32, tag="mx")
            ss = wp.tile([P, 64], f32, tag="ss")
            tmp = wp.tile([P, 64], f32, tag="tmp")
            tmp2 = wp.tile([P, 64], f32, tag="tmp2")
            djk = wp.tile([P, 64], f32, tag="djk")
            normg = wp.tile([P, 64], f32, tag="normg")
            dkg = wp.tile([P, 64], f32, tag="dkg")
            rv = wp.tile([P, NB], f32, tag="rv")
            ejd = wp.tile([P, NB * F], f32, tag="ejd")

            dJ = dd[:].unsqueeze(2).broadcast_to((P, NB, NB))
            dK = dd[:].unsqueeze(1).broadcast_to((P, NB, NB))
            g3 = lambda a: a[:].rearrange("p (a b) -> p a b", b=NB)
            nc.vector.tensor_tensor(out=g3(mn), in0=dJ, in1=dK, op=ALU.min)
            nc.vector.tensor_tensor(out=g3(mx), in0=dJ, in1=dK, op=ALU.max)

            xs3 = xs[:].rearrange("p (j c) -> p j c", c=4)
            for ci in range(3):
                cJ = xs3[:, :, ci : ci + 1].broadcast_to((P, NB, NB))
                cK = xs3[:, :, ci : ci + 1].transpose((0, 2, 1)).broadcast_to((P, NB, NB))
                nc.vector.tensor_tensor(out=g3(tmp), in0=cJ, in1=cK, op=ALU.subtract)
                if ci == 0:
                    nc.vector.tensor_tensor(out=g3(ss), in0=g3(tmp), in1=g3(tmp), op=ALU.mult)
                else:
                    nc.vector.tensor_tensor(out=g3(tmp2), in0=g3(tmp), in1=g3(tmp), op=ALU.mult)
                    nc.vector.tensor_tensor(out=g3(ss), in0=g3(ss), in1=g3(tmp2), op=ALU.add)
            nc.scalar.sqrt(djk[:], ss[:])
            # norm = (d_jk - (mx - mn)) / (2 * mn)
            nc.vector.tensor_tensor(out=tmp[:], in0=mx[:], in1=mn[:], op=ALU.subtract)
            nc.vector.tensor_tensor(out=tmp2[:], in0=djk[:], in1=tmp[:], op=ALU.subtract)
            nc.vector.tensor_tensor(out=tmp[:], in0=mn[:], in1=mn[:], op=ALU.add)
            nc.vector.reciprocal(out=djk[:], in_=tmp[:])
            nc.vector.tensor_tensor(out=normg[:], in0=tmp2[:], in1=djk[:], op=ALU.mult)

            # d_ik grid materialized (for the off-diagonal copy)
            nc.scalar.copy(out=g3(dkg), in_=dK)

            # ejd = emb_j / d  (shared by the ej and ek blocks)
            nc.vector.reciprocal(out=rv[:], in_=dd[:])
            nc.vector.tensor_tensor(
                out=ejd[:].rearrange("p (j f) -> p j f", f=F),
                in0=pej[:].rearrange("p (j f) -> p j f", f=F),
                in1=rv[:].unsqueeze(2).broadcast_to((P, NB, F)),
                op=ALU.mult,
            )

            # ---- assemble the (128, 56*195) output tile ----
            big = bp.tile([P, ROW], f32, tag="big")
            bigap = big[:]
            # col0: d_ij (constant within each group of 7 pairs)
            nc.scalar.copy(
                out=_ap_of(bigap, 0, [[7 * FEAT, NB], [FEAT, 7]]),
                in_=dd[:].unsqueeze(2).broadcast_to((P, NB, 7)),
            )
            # col1: d_ik via off-diagonal view (flat[1:64] as (7,9)[:, :8])
            nc.scalar.copy(
                out=_ap_of(bigap, 1, [[8 * FEAT, 7], [FEAT, 8]]),
                in_=_ap_of(dkg[:], 1, [[9, 7], [1, 8]]),
            )
            # col2: d_jk_norm off-diagonal
            nc.vector.tensor_copy(
                out=_ap_of(bigap, 2, [[8 * FEAT, 7], [FEAT, 8]]),
                in_=_ap_of(normg[:], 1, [[9, 7], [1, 8]]),
            )
            # ei block: emb_i broadcast to all 56 pairs (ACT: DVE is the
            # busier engine; the PSUM read is fine from ACT)
            nc.scalar.copy(
                out=_ap_of(bigap, 3, [[FEAT, NPAIR], [1, F]]),
                in_=_ap_of(pei[:], 0, [[0, NPAIR], [1, F]]),
            )
            # ej block: ejd[j] broadcast over the 7 pairs of group j
            nc.vector.tensor_copy(
                out=_ap_of(bigap, 67, [[7 * FEAT, NB], [FEAT, 7], [1, F]]),
                in_=_ap_of(ejd[:], 0, [[F, NB], [0, 7], [1, F]]),
            )
            # ek block: within group j, k runs over {0..7}\{j} as two runs
            for j in range(NB):
                if j > 0:
                    nc.scalar.copy(
                        out=_ap_of(bigap, (7 * j) * FEAT + 131, [[FEAT, j], [1, F]]),
                        in_=_ap_of(ejd[:], 0, [[F, j], [1, F]]),
                    )
                if j < 7:
                    nc.vector.tensor_copy(
                        out=_ap_of(bigap, (7 * j + j) * FEAT + 131, [[FEAT, 7 - j], [1, F]]),
                        in_=_ap_of(ejd[:], (j + 1) * F, [[F, 7 - j], [1, F]]),
                    )

            nc.sync.dma_start(out=out_d[r0 : r0 + P, :], in_=big[:])


_NC_CACHE = None


def _get_nc():
    global _NC_CACHE
    if _NC_CACHE is not None:
        return _NC_CACHE
    nc = bacc.Bacc("TRN2", target_bir_lowering=False, debug=False, num_devices=N_CORES)
    tj_d = nc.dram_tensor("tj", [NCL, NB], i32, kind="ExternalInput").ap()
    dd_d = nc.dram_tensor("dd", [NCL, NB], f32, kind="ExternalInput").ap()
    ai_d = nc.dram_tensor("ai", [NCL, 1], i32, kind="ExternalInput").ap()
    pk_d = nc.dram_tensor("pk", [N_ATOMS, 4], f32, kind="ExternalInput").ap()
    em_d = nc.dram_tensor("em", [16, F], f32, kind="ExternalInput").ap()
    bd_d = nc.dram_tensor("bd", [P, NB * F], f32, kind="ExternalInput").ap()
    cst_d = nc.dram_tensor("cst", [P, P + 1], f32, kind="ExternalInput").ap()
    out_d = nc.dram_tensor("ang", [NCL, ROW], f32, kind="ExternalOutput").ap()
    with tile.TileContext(nc) as tc:
        _build_body(nc, tc, tj_d, dd_d, ai_d, pk_d, em_d, bd_d, cst_d, out_d)
    nc.compile()
    _NC_CACHE = nc
    return nc


def kernel(nNeigh, atom_i_idx, atom_j_idx, dist_ij, atoms_xyz, atoms_long,
           embed_table, trace=False, tmpdir=None, **_unused):
    atom_i_idx = np.asarray(atom_i_idx)
    aj = np.asarray(atom_j_idx).astype(np.int32).reshape(N_CENTER, NB)
    dist = np.asarray(dist_ij).astype(np.float32).reshape(N_CENTER, NB)
    ai = np.asarray(atom_i_idx).astype(np.int32).reshape(N_CENTER, 1)
    xyz = np.asarray(atoms_xyz).astype(np.float32)
    spec = np.asarray(atoms_long)[:, 1].astype(np.int32)
    em = np.ascontiguousarray(np.asarray(embed_table).astype(np.float32))

    # packed per-atom table: [x, y, z, species (as float value)]
    pk = np.empty((N_ATOMS, 4), np.float32)
    pk[:, :3] = xyz
    pk[:, 3] = spec.astype(np.float32)

    # block-diagonal embed table (8 copies on the diagonal) for the one-hot
    # matmul, and [identity | iota16] constants
    bd = np.zeros((P, NB * F), np.float32)
    for j in range(NB):
        bd[16 * j : 16 * j + 16, F * j : F * j + F] = em
    cst = np.zeros((P, P + 1), np.float32)
    cst[:, :P] = np.eye(P, dtype=np.float32)
    cst[:, P] = np.arange(P, dtype=np.float32) % 16

    # pad the center dim to 8*1280 and shard
    def pad(a, fill):
        out = np.full((NPAD,) + a.shape[1:], fill, a.dtype)
        out[:N_CENTER] = a
        return out

    aj_p, dist_p, ai_p = pad(aj, 0), pad(dist, 1.0), pad(ai, 0)

    in_maps = []
    for c in range(N_CORES):
        s = slice(c * NCL, (c + 1) * NCL)
        in_maps.append({
            "tj": np.ascontiguousarray(aj_p[s]),
            "dd": np.ascontiguousarray(dist_p[s]),
            "ai": np.ascontiguousarray(ai_p[s]),
            "pk": pk,
            "em": em,
            "bd": bd,
            "cst": cst,
        })

    nc = _get_nc()
    res = run_bass_kernel_spmd(
        nc, in_maps, core_ids=list(range(N_CORES)), trace=trace, tmpdir=tmpdir
    )
    ang = np.concatenate([res.results[c]["ang"] for c in range(N_CORES)], axis=0)
    ang = ang[:N_CENTER].reshape(N_CENTER, NPAIR, FEAT)
    out = (atom_i_idx.reshape(-1), ang)
    if trace:
        return out, res
    return out
